# revision 1
# baseline (speedup 1.0000x reference)
"""Trainium2 Bass kernel for nn_Attention (Bahdanau-style additive attention).

Reference computation:
    enc = encoder_outputs.transpose(1, 0, 2)            # [B, S, 2H]
    e_proj = enc @ w_e.T                                # [B, S, H]
    energy = tanh(h_proj[:, None, :] + e_proj + b)      # [B, S, H]
    att = energy @ v_w                                  # [B, S]
    out = softmax(att, axis=1)

Sharding: data-parallel over batch, 4 batch rows per core on 8 cores.
Per-core pipeline (all heavy compute in bf16 on the PE):
  - the encoder slice is DMA-transposed (xbar) from DRAM bf16 [S, 2H]
    into SBUF [128, 16, 512] tiles so the contraction dim (e) lands on
    partitions; one tile per 512 source positions
  - main matmul: psum[s-tile(128), h(512)] = sum_e enc^T chunk (the PE
    stationary, reused for both h-groups) @ w_e^T chunk; 16 e-chunks
    accumulate per bank
  - epilogue on the otherwise-idle Vector/Scalar engines:
    DVE adds the host-precomputed broadcast bias c_b = h_proj + attn_b,
    ACT applies tanh, DVE multiplies by v_w and reduces over h (free
    axis) straight into the attention logit column
  - batch row 0 ramps h-slice segments as its transposes land so the PE
    starts ~16us in; subsequent rows prefetch transposes inside the
    previous row's compute
h_proj ([32,1024] @ [1024,1024]) and the final softmax over [32, 2048]
are tiny and run on the host in fp32.
"""

import sys

try:
    import concourse.bass as bass  # noqa: F401
except ImportError:
    sys.path.insert(0, "/opt/trn_rl_repo")

import numpy as np
import ml_dtypes

import concourse.bacc as bacc
import concourse.mybir as mybir
import concourse.tile as tile
from concourse.bass_utils import run_bass_kernel_spmd

HID = 1024
BATCH = 32
SRC_LEN = 2048

N_CORES = 8
B_LOC = BATCH // N_CORES      # 4
E = 2 * HID                   # 2048
SG = 512                      # s per encoder transpose tile
N_SG = SRC_LEN // SG          # 4
N_EC = E // 128               # 16 e-chunks
N_HC = HID // 128             # 8 h-slices
N_ST = SRC_LEN // 128         # 16 s-tiles per batch row
HG = 512                      # h per psum bank
N_HG = HID // HG              # 2 h-groups

f32 = mybir.dt.float32
bf16 = mybir.dt.bfloat16

_NC_CACHE = {}


def _build():
    nc = bacc.Bacc(
        "TRN2", target_bir_lowering=False, debug=False, num_devices=N_CORES
    )
    enc = nc.declare_dram_parameter("enc", [B_LOC, SRC_LEN, E], bf16, isOutput=False)
    wT = nc.declare_dram_parameter("wT", [N_HC, 128, N_EC * 128], bf16, isOutput=False)
    cbb = nc.declare_dram_parameter("cbb", [B_LOC, 128, HID], f32, isOutput=False)
    vb = nc.declare_dram_parameter("vb", [128, HID], bf16, isOutput=False)
    # [b, p, st]: logit(b, st*128 + p)
    att = nc.declare_dram_parameter("att", [B_LOC, 128, N_ST], f32, isOutput=True)

    with tile.TileContext(nc) as tc:
        with (
            tc.tile_pool(name="const", bufs=1) as const_pool,
            tc.tile_pool(name="cbbp", bufs=2) as cbb_pool,
            tc.tile_pool(name="encT", bufs=6) as encT_pool,
            tc.tile_pool(name="tanhE", bufs=18) as te_pool,
            tc.tile_pool(name="scratch", bufs=3) as sc_pool,
            tc.tile_pool(name="attsb", bufs=1) as att_pool,
            tc.tile_pool(name="psum", bufs=5, space="PSUM") as psum_pool,
            tc.tile_pool(name="psumr", bufs=3, space="PSUM") as psumr_pool,
        ):
            w_sb = const_pool.tile([128, N_HC, N_EC, 128], bf16)
            vb_sb = const_pool.tile([128, HID], bf16)
            att_sb = att_pool.tile([128, B_LOC * N_ST], f32)

            def load_w_slice(hs):
                nc.sync.dma_start(
                    w_sb[:, hs].rearrange("p c h -> p (c h)"), wT[hs]
                )

            cbb_sbs = [None] * B_LOC

            def load_cbb(b):
                t = cbb_pool.tile([128, HID], f32, tag="cbb", name=f"cbb_{b}")
                nc.sync.dma_start(t[:], cbb[b])
                cbb_sbs[b] = t

            def transpose_group(b, sg):
                encT = encT_pool.tile(
                    [128, N_EC, SG], bf16, tag="encT", name=f"encT_{b}_{sg}"
                )
                nc.sync.dma_start(
                    out=encT[:],
                    in_=enc[b, sg * SG:(sg + 1) * SG, :],
                    transpose=True,
                )
                return encT

            # startup DMA order on the serial chain: just enough weight
            # for the first ramp segment before the first transpose
            load_w_slice(0)
            load_w_slice(1)

            # warmup tanh for the ACT LUT-table dependency
            warm = const_pool.tile([128, 1], f32)
            nc.scalar.activation(
                warm[:], w_sb[:, 0, 0, 0:1], mybir.ActivationFunctionType.Tanh
            )

            def lhs_enc(encT, st, c):
                j = st % N_SG
                return encT[:, c, j * 128:(j + 1) * 128]

            def epilogue_half(b, st, ps, hg, tanhE):
                # energy = tanh(psum + c_b), half h-group at a time
                pre = sc_pool.tile(
                    [128, HG], bf16, tag="pre", name=f"pre_{b}_{st}_{hg}"
                )
                nc.vector.tensor_add(
                    out=pre[:],
                    in0=ps[:],
                    in1=cbb_sbs[b][:, hg * HG:(hg + 1) * HG],
                )
                nc.scalar.activation(
                    tanhE[:, hg * HG:(hg + 1) * HG], pre[:],
                    mybir.ActivationFunctionType.Tanh,
                )

            def vdot(b, st, tanhE):
                # energy * v then reduce over h (free axis), both on DVE
                outj = sc_pool.tile(
                    [128, HID], bf16, tag="ttr", name=f"ttr_{b}_{st}"
                )
                nc.vector.tensor_mul(out=outj[:], in0=tanhE[:], in1=vb_sb[:])
                nc.vector.tensor_reduce(
                    att_sb[:, b * N_ST + st:b * N_ST + st + 1],
                    outj[:],
                    mybir.AxisListType.X,
                    mybir.AluOpType.add,
                )

            # ---- batch row 0: ramp as transposes land ----
            # pass 1: h-group 0 per s-tile in two h-slice-pair segments;
            # each segment is one accumulation group on the bank and is
            # drained before the next segment reopens the zero region
            encTs = []
            tanhEs = {}
            for sg in range(N_SG):
                encTs.append(transpose_group(0, sg))
                if sg == 0:
                    load_cbb(0)
                    load_w_slice(2)
                    load_w_slice(3)
                elif sg == 1:
                    for hs in range(4, 6):
                        load_w_slice(hs)
                elif sg == 2:
                    for hs in range(6, N_HC):
                        load_w_slice(hs)
            nc.sync.dma_start(vb_sb[:], vb[:])
            for sg in range(N_SG):
                for st in range(sg * N_SG, (sg + 1) * N_SG):
                    tanhEs[st] = te_pool.tile(
                        [128, HID], bf16, tag="te", name=f"te0_{st}"
                    )
                for seg in range(2):
                    for st in range(sg * N_SG, (sg + 1) * N_SG):
                        ps = psumr_pool.tile(
                            [128, 256], f32, tag="psr", name=f"psr_{st}_{seg}"
                        )
                        for c in range(N_EC):
                            for hh in range(2):
                                hs = seg * 2 + hh
                                nc.tensor.matmul(
                                    ps[:, hh * 128:(hh + 1) * 128],
                                    lhsT=lhs_enc(encTs[sg], st, c),
                                    rhs=w_sb[:, hs, c, :],
                                    start=(c == 0 and hh == 0),
                                    stop=(c == N_EC - 1 and hh == 1),
                                )
                        pre = sc_pool.tile(
                            [128, 256], bf16, tag="prer", name=f"prer_{st}_{seg}"
                        )
                        nc.vector.tensor_add(
                            out=pre[:],
                            in0=ps[:],
                            in1=cbb_sbs[0][:, seg * 256:(seg + 1) * 256],
                        )
                        nc.scalar.activation(
                            tanhEs[st][:, seg * 256:(seg + 1) * 256], pre[:],
                            mybir.ActivationFunctionType.Tanh,
                        )
            # pass 2: h-group 1 + v-dot per s-tile; prefetch b1's tiles
            encTs_next = []
            for st in range(N_ST):
                sg = st // N_SG
                ps1 = psum_pool.tile([128, HG], f32, tag="ps", name=f"ps1_{st}")
                for c in range(N_EC):
                    nc.tensor.matmul(
                        ps1[:],
                        lhsT=lhs_enc(encTs[sg], st, c),
                        rhs=w_sb[:, 4:8, c, :],
                        start=(c == 0),
                        stop=(c == N_EC - 1),
                    )
                if st == 0:
                    encTs_next.append(transpose_group(1, 0))
                    load_cbb(1)
                elif st in (2, 5, 9):
                    encTs_next.append(transpose_group(1, len(encTs_next)))
                epilogue_half(0, st, ps1, 1, tanhEs[st])
                vdot(0, st, tanhEs[st])
            nc.sync.dma_start(att[0], att_sb[:, 0:N_ST])

            # ---- batch rows 1..3: steady state ----
            for b in range(1, B_LOC):
                encTs = encTs_next
                encTs_next = []
                for st in range(N_ST):
                    sg = st // N_SG
                    if b < B_LOC - 1:
                        if st == 1:
                            encTs_next.append(transpose_group(b + 1, 0))
                            load_cbb(b + 1)
                        elif st in (3, 6, 10):
                            encTs_next.append(transpose_group(b + 1, len(encTs_next)))
                    ps = [
                        psum_pool.tile(
                            [128, HG], f32, tag="ps", name=f"ps_{b}_{st}_{g}"
                        )
                        for g in range(N_HG)
                    ]
                    for c in range(N_EC):
                        for hg in range(N_HG):
                            nc.tensor.matmul(
                                ps[hg][:],
                                lhsT=lhs_enc(encTs[sg], st, c),
                                rhs=w_sb[:, hg * 4:(hg + 1) * 4, c, :],
                                start=(c == 0),
                                stop=(c == N_EC - 1),
                            )
                    tanhE = te_pool.tile(
                        [128, HID], bf16, tag="te", name=f"te_{b}_{st}"
                    )
                    for hg in range(N_HG):
                        epilogue_half(b, st, ps[hg], hg, tanhE)
                    vdot(b, st, tanhE)
                nc.sync.dma_start(att[b], att_sb[:, b * N_ST:(b + 1) * N_ST])
    nc.compile()
    return nc


def _get_nc():
    if "nc" not in _NC_CACHE:
        _NC_CACHE["nc"] = _build()
    return _NC_CACHE["nc"]


def kernel(hidden, encoder_outputs, attn_w, attn_b, v_w, _trace=False):
    hidden = np.asarray(hidden, dtype=np.float32)
    encoder_outputs = np.asarray(encoder_outputs, dtype=np.float32)
    attn_w = np.asarray(attn_w, dtype=np.float32)
    attn_b = np.asarray(attn_b, dtype=np.float32)
    v_w = np.asarray(v_w, dtype=np.float32)

    c_b = hidden @ attn_w[:, :HID].T + attn_b          # [B, H] fp32
    w_e = attn_w[:, HID:]                              # [H, E]
    wT_bf = np.ascontiguousarray(
        w_e.reshape(N_HC, 128, N_EC, 128).transpose(0, 3, 2, 1)
        .reshape(N_HC, 128, N_EC * 128)
    ).astype(ml_dtypes.bfloat16)
    vb_dev = np.ascontiguousarray(
        np.broadcast_to(v_w[None, :], (128, HID))
    ).astype(ml_dtypes.bfloat16)

    nc = _get_nc()
    in_maps = []
    for core in range(N_CORES):
        b0 = core * B_LOC
        enc_bf = np.ascontiguousarray(
            encoder_outputs[:, b0:b0 + B_LOC, :].transpose(1, 0, 2)
        ).astype(ml_dtypes.bfloat16)
        cbb_dev = np.ascontiguousarray(
            np.broadcast_to(c_b[b0:b0 + B_LOC, None, :], (B_LOC, 128, HID))
        ).astype(np.float32)
        in_maps.append(
            {"enc": enc_bf, "wT": wT_bf, "cbb": cbb_dev, "vb": vb_dev}
        )

    res = run_bass_kernel_spmd(
        nc, in_maps, core_ids=list(range(N_CORES)), trace=_trace
    )
    if _trace:
        _NC_CACHE["last_result"] = res

    att = np.concatenate(
        [
            res.results[c]["att"].transpose(0, 2, 1).reshape(B_LOC, SRC_LEN)
            for c in range(N_CORES)
        ],
        axis=0,
    )  # [B, S] logits

    m = att.max(axis=1, keepdims=True)
    e = np.exp(att - m)
    out = e / e.sum(axis=1, keepdims=True)
    return out.astype(np.float32)



# revision 2
# speedup vs baseline: 1.1341x; 1.1341x over previous
"""Trainium2 Bass kernel for nn_Attention (Bahdanau-style additive attention).

Reference computation:
    enc = encoder_outputs.transpose(1, 0, 2)            # [B, S, 2H]
    e_proj = enc @ w_e.T                                # [B, S, H]
    energy = tanh(h_proj[:, None, :] + e_proj + b)      # [B, S, H]
    att = energy @ v_w                                  # [B, S]
    out = softmax(att, axis=1)

Sharding: data-parallel over batch, 4 batch rows per core on 8 cores.

Per-core pipeline, mixed-precision contraction over E = 2048:
  - the first NC16*128 contraction rows run as fp16 matmuls (full
    precision), the remaining N8 pairs of 128-row chunks run as fp8
    (e4m3) matmuls in DoubleRow perf mode at 2x PE throughput
  - enc is pre-transposed and quantized on the host into partition-major
    [e, chunk, s] lines so every DMA is a plain (non-transpose) load
  - w_e and c_b are pre-scaled by WS=64 so the fp8 weights stay in the
    e4m3 normal range; the tanh activation applies scale=1/WS to undo it
  - epilogue per 128-position s-tile on the otherwise idle engines:
    DVE adds the host-precomputed broadcast bias c_b*WS, ACT applies
    tanh(x/WS), DVE multiplies by v_w and reduces over h into the logit
  - enc streams on the SP DMA queue (row 0 in four 512-position slabs
    so the PE starts early; rows 1-3 as single whole-row loads
    prefetched during the previous row), small tensors on the ACT queue
h_proj ([32,1024] @ [1024,1024]) and the final softmax over [32, 2048]
are tiny and run on the host in fp32.
"""

import sys

try:
    import concourse.bass as bass  # noqa: F401
except ImportError:
    sys.path.insert(0, "/opt/trn_rl_repo")

import numpy as np
import ml_dtypes

import concourse.bacc as bacc
import concourse.mybir as mybir
import concourse.tile as tile
from concourse.bass_utils import run_bass_kernel_spmd

HID = 1024
BATCH = 32
SRC_LEN = 2048

N_CORES = 8
B_LOC = BATCH // N_CORES      # 4
E = 2 * HID                   # 2048
N_EC = E // 128               # 16 e-chunks of 128
N8 = 4                        # fp8 DoubleRow chunk-pairs (2*N8 chunks)
NC16 = N_EC - 2 * N8          # fp16 chunks
N_ST = SRC_LEN // 128         # 16 s-tiles per batch row
HG = 512                      # h per psum bank
N_HG = HID // HG              # 2 h-groups
WS = 64.0                     # weight/bias pre-scale (fp8 range)
SG = 512                      # row-0 DMA slab width in s
N_SG = SRC_LEN // SG          # 4

f32 = mybir.dt.float32
fp16 = mybir.dt.float16
fp8 = mybir.dt.float8e4

_NC_CACHE = {}


def _build():
    nc = bacc.Bacc(
        "TRN2", target_bir_lowering=False, debug=False, num_devices=N_CORES
    )
    enc16 = nc.declare_dram_parameter(
        "enc16", [B_LOC, 128, NC16, SRC_LEN], fp16, isOutput=False
    )
    enc8 = nc.declare_dram_parameter(
        "enc8", [B_LOC, 128, 2 * N8, SRC_LEN], fp8, isOutput=False
    )
    w16 = nc.declare_dram_parameter("w16", [128, NC16, HID], fp16, isOutput=False)
    w8 = nc.declare_dram_parameter("w8", [128, 2 * N8, HID], fp8, isOutput=False)
    cbb = nc.declare_dram_parameter("cbb", [B_LOC, 128, HID], f32, isOutput=False)
    vb = nc.declare_dram_parameter("vb", [128, HID], fp16, isOutput=False)
    # [b, p, st]: logit(b, st*128 + p)
    att = nc.declare_dram_parameter("att", [B_LOC, 128, N_ST], f32, isOutput=True)

    with tile.TileContext(nc) as tc:
        with (
            tc.tile_pool(name="const", bufs=1) as const_pool,
            tc.tile_pool(name="e16p", bufs=2) as e16_pool,
            tc.tile_pool(name="e8p", bufs=2) as e8_pool,
            tc.tile_pool(name="cbbp", bufs=2) as cbb_pool,
            tc.tile_pool(name="prep", bufs=3) as pre_pool,
            tc.tile_pool(name="tep", bufs=3) as te_pool,
            tc.tile_pool(name="ttp", bufs=2) as tt_pool,
            tc.tile_pool(name="attsb", bufs=1) as att_pool,
            tc.tile_pool(name="psum", bufs=6, space="PSUM") as psum_pool,
        ):
            w16_sb = const_pool.tile([128, NC16, HID], fp16)
            w8_sb = const_pool.tile([128, 2 * N8, HID], fp8)
            vb_sb = const_pool.tile([128, HID], fp16)
            att_sb = att_pool.tile([128, B_LOC * N_ST], f32)

            # small/const tensors on the ACT hwdge queue so they don't
            # serialize behind the enc stream on SP
            nc.scalar.dma_start(w16_sb[:], w16[:])
            nc.scalar.dma_start(w8_sb[:], w8[:])
            nc.scalar.dma_start(vb_sb[:], vb[:])

            cbb_sbs = [None] * B_LOC

            def load_cbb(b):
                t = cbb_pool.tile([128, HID], f32, tag="cbb", name=f"cbb_{b}")
                nc.scalar.dma_start(t[:], cbb[b])
                cbb_sbs[b] = t

            load_cbb(0)

            # warmup tanh for the ACT LUT-table dependency
            warm = const_pool.tile([128, 1], f32)
            nc.scalar.activation(
                warm[:], vb_sb[:, 0:1], mybir.ActivationFunctionType.Tanh
            )

            enc16_sbs = [None] * B_LOC
            enc8_sbs = [None] * B_LOC

            def alloc_row(b):
                enc16_sbs[b] = e16_pool.tile(
                    [128, NC16, SRC_LEN], fp16, tag="e16", name=f"e16_{b}"
                )
                enc8_sbs[b] = e8_pool.tile(
                    [128, 2 * N8, SRC_LEN], fp8, tag="e8", name=f"e8_{b}"
                )

            def load_row(b):
                nc.sync.dma_start(enc16_sbs[b][:], enc16[b])
                nc.sync.dma_start(enc8_sbs[b][:], enc8[b])

            # row 0 in s-slabs so the PE can start after the first slab
            alloc_row(0)
            for sg in range(N_SG):
                nc.sync.dma_start(
                    enc16_sbs[0][:, :, sg * SG:(sg + 1) * SG],
                    enc16[0, :, :, sg * SG:(sg + 1) * SG],
                )
                nc.sync.dma_start(
                    enc8_sbs[0][:, :, sg * SG:(sg + 1) * SG],
                    enc8[0, :, :, sg * SG:(sg + 1) * SG],
                )
            # row 1 goes straight after on the other buffer
            alloc_row(1)
            load_row(1)
            load_cbb(1)

            for b in range(B_LOC):
                for st in range(N_ST):
                    # prefetch row b+2 into the buffer row b just freed
                    if b + 2 < B_LOC and st == 2:
                        alloc_row(b + 2)
                        load_row(b + 2)
                        load_cbb(b + 2)
                    sl = slice(st * 128, (st + 1) * 128)
                    ps = [
                        psum_pool.tile(
                            [128, HG], f32, tag="ps", name=f"ps_{b}_{st}_{g}"
                        )
                        for g in range(N_HG)
                    ]
                    for hg in range(N_HG):
                        hsl = slice(hg * HG, (hg + 1) * HG)
                        for c in range(NC16):
                            nc.tensor.matmul(
                                ps[hg][:],
                                lhsT=enc16_sbs[b][:, c, sl],
                                rhs=w16_sb[:, c, hsl],
                                start=(c == 0),
                                stop=False,
                            )
                        for j in range(N8):
                            nc.tensor.matmul(
                                ps[hg][:],
                                lhsT=enc8_sbs[b][:, 2 * j:2 * j + 2, sl],
                                rhs=w8_sb[:, 2 * j:2 * j + 2, hsl],
                                start=False,
                                stop=(j == N8 - 1),
                                perf_mode=mybir.MatmulPerfMode.DoubleRow,
                            )
                    tanhE = te_pool.tile(
                        [128, HID], fp16, tag="te", name=f"te_{b}_{st}"
                    )
                    for hg in range(N_HG):
                        hsl = slice(hg * HG, (hg + 1) * HG)
                        pre = pre_pool.tile(
                            [128, HG], f32, tag="pre", name=f"pre_{b}_{st}_{hg}"
                        )
                        nc.vector.tensor_add(
                            out=pre[:], in0=ps[hg][:], in1=cbb_sbs[b][:, hsl]
                        )
                        nc.scalar.activation(
                            tanhE[:, hsl], pre[:],
                            mybir.ActivationFunctionType.Tanh,
                            scale=1.0 / WS,
                        )
                    tt = tt_pool.tile([128, HID], fp16, tag="tt", name=f"tt_{b}_{st}")
                    nc.vector.tensor_mul(out=tt[:], in0=tanhE[:], in1=vb_sb[:])
                    nc.vector.tensor_reduce(
                        att_sb[:, b * N_ST + st:b * N_ST + st + 1],
                        tt[:],
                        mybir.AxisListType.X,
                        mybir.AluOpType.add,
                    )
                nc.scalar.dma_start(att[b], att_sb[:, b * N_ST:(b + 1) * N_ST])
    nc.compile()
    return nc


def _get_nc():
    if "nc" not in _NC_CACHE:
        _NC_CACHE["nc"] = _build()
    return _NC_CACHE["nc"]


def kernel(hidden, encoder_outputs, attn_w, attn_b, v_w, _trace=False):
    hidden = np.asarray(hidden, dtype=np.float32)
    encoder_outputs = np.asarray(encoder_outputs, dtype=np.float32)
    attn_w = np.asarray(attn_w, dtype=np.float32)
    attn_b = np.asarray(attn_b, dtype=np.float32)
    v_w = np.asarray(v_w, dtype=np.float32)

    c_b = (hidden @ attn_w[:, :HID].T + attn_b) * WS   # [B, H] fp32, pre-scaled
    w_e = attn_w[:, HID:]                              # [H, E]
    # [E, H] -> [chunk, e, h] -> partition-major [e, chunk, h]
    w_t = (w_e.T * WS).reshape(N_EC, 128, HID)
    w16_dev = np.ascontiguousarray(
        w_t[:NC16].transpose(1, 0, 2)
    ).astype(np.float16)
    w8_dev = np.ascontiguousarray(
        w_t[NC16:].transpose(1, 0, 2)
    ).astype(ml_dtypes.float8_e4m3)
    vb_dev = np.ascontiguousarray(
        np.broadcast_to(v_w[None, :], (128, HID))
    ).astype(np.float16)

    nc = _get_nc()
    in_maps = []
    for core in range(N_CORES):
        b0 = core * B_LOC
        # enc[:, b, :] is [S, E]; make [e, chunk, s] lines per batch row
        e16_rows = np.empty((B_LOC, 128, NC16, SRC_LEN), dtype=np.float16)
        e8_rows = np.empty((B_LOC, 128, 2 * N8, SRC_LEN), dtype=ml_dtypes.float8_e4m3)
        for b in range(B_LOC):
            ect = encoder_outputs[:, b0 + b, :].T.reshape(N_EC, 128, SRC_LEN)
            e16_rows[b] = ect[:NC16].transpose(1, 0, 2)
            e8_rows[b] = ect[NC16:].transpose(1, 0, 2)
        cbb_dev = np.ascontiguousarray(
            np.broadcast_to(c_b[b0:b0 + B_LOC, None, :], (B_LOC, 128, HID))
        ).astype(np.float32)
        in_maps.append(
            {
                "enc16": e16_rows,
                "enc8": e8_rows,
                "w16": w16_dev,
                "w8": w8_dev,
                "cbb": cbb_dev,
                "vb": vb_dev,
            }
        )

    res = run_bass_kernel_spmd(
        nc, in_maps, core_ids=list(range(N_CORES)), trace=_trace
    )
    if _trace:
        _NC_CACHE["last_result"] = res

    att = np.concatenate(
        [
            res.results[c]["att"].transpose(0, 2, 1).reshape(B_LOC, SRC_LEN)
            for c in range(N_CORES)
        ],
        axis=0,
    )  # [B, S] logits

    m = att.max(axis=1, keepdims=True)
    e = np.exp(att - m)
    out = e / e.sum(axis=1, keepdims=True)
    return out.astype(np.float32)


# revision 3
# speedup vs baseline: 1.2094x; 1.0664x over previous
"""Trainium2 Bass kernel for nn_Attention (Bahdanau-style additive attention).

Reference computation:
    enc = encoder_outputs.transpose(1, 0, 2)            # [B, S, 2H]
    e_proj = enc @ w_e.T                                # [B, S, H]
    energy = tanh(h_proj[:, None, :] + e_proj + b)      # [B, S, H]
    att = energy @ v_w                                  # [B, S]
    out = softmax(att, axis=1)

Sharding: data-parallel over batch, 4 batch rows per core on 8 cores.

Per-core pipeline, mixed-precision contraction over E = 2048:
  - the first NC16*128 contraction rows run as fp16 matmuls (full
    precision), the remaining N8 pairs of 128-row chunks run as fp8
    (e4m3) matmuls in DoubleRow perf mode at 2x PE throughput
  - enc is pre-transposed and quantized on the host into partition-major
    [e, chunk, s] lines so every DMA is a plain (non-transpose) load
  - w_e and c_b are pre-scaled by WS=64 so the fp8 weights stay in the
    e4m3 normal range; the tanh activation applies scale=1/WS to undo it
  - epilogue per 128-position s-tile on the otherwise idle engines:
    DVE adds the host-precomputed broadcast bias c_b*WS, ACT applies
    tanh(x/WS), DVE multiplies by v_w and reduces over h into the logit
  - enc streams on the SP DMA queue (row 0 in four 512-position slabs
    so the PE starts early; rows 1-3 as single whole-row loads
    prefetched during the previous row), small tensors on the ACT queue
h_proj ([32,1024] @ [1024,1024]) and the final softmax over [32, 2048]
are tiny and run on the host in fp32.
"""

import sys

try:
    import concourse.bass as bass  # noqa: F401
except ImportError:
    sys.path.insert(0, "/opt/trn_rl_repo")

import numpy as np
import ml_dtypes

import concourse.bacc as bacc
import concourse.mybir as mybir
import concourse.tile as tile
from concourse.bass_utils import run_bass_kernel_spmd

HID = 1024
BATCH = 32
SRC_LEN = 2048

N_CORES = 8
B_LOC = BATCH // N_CORES      # 4
E = 2 * HID                   # 2048
N_EC = E // 128               # 16 e-chunks of 128
N8 = 4                        # fp8 DoubleRow chunk-pairs (2*N8 chunks)
NC16 = N_EC - 2 * N8          # fp16 chunks
N_ST = SRC_LEN // 128         # 16 s-tiles per batch row
HG = 512                      # h per psum bank
N_HG = HID // HG              # 2 h-groups
WS = 64.0                     # weight/bias pre-scale (fp8 range)
SG = 512                      # row-0 DMA slab width in s
N_SG = SRC_LEN // SG          # 4

f32 = mybir.dt.float32
fp16 = mybir.dt.float16
fp8 = mybir.dt.float8e4

_NC_CACHE = {}


def _build():
    nc = bacc.Bacc(
        "TRN2", target_bir_lowering=False, debug=False, num_devices=N_CORES
    )
    enc16 = nc.declare_dram_parameter(
        "enc16", [B_LOC, 128, NC16, SRC_LEN], fp16, isOutput=False
    )
    enc8 = nc.declare_dram_parameter(
        "enc8", [B_LOC, 128, 2 * N8, SRC_LEN], fp8, isOutput=False
    )
    w16 = nc.declare_dram_parameter("w16", [128, NC16, HID], fp16, isOutput=False)
    w8 = nc.declare_dram_parameter("w8", [128, 2 * N8, HID], fp8, isOutput=False)
    cbb = nc.declare_dram_parameter("cbb", [B_LOC, 128, HID], f32, isOutput=False)
    vb = nc.declare_dram_parameter("vb", [128, HID], fp16, isOutput=False)
    # [b, p, st]: logit(b, st*128 + p)
    att = nc.declare_dram_parameter("att", [B_LOC, 128, N_ST], f32, isOutput=True)

    with tile.TileContext(nc) as tc:
        with (
            tc.tile_pool(name="const", bufs=1) as const_pool,
            tc.tile_pool(name="e16p", bufs=2) as e16_pool,
            tc.tile_pool(name="e8p", bufs=2) as e8_pool,
            tc.tile_pool(name="cbbp", bufs=2) as cbb_pool,
            tc.tile_pool(name="prep", bufs=3) as pre_pool,
            tc.tile_pool(name="tep", bufs=3) as te_pool,
            tc.tile_pool(name="ttp", bufs=2) as tt_pool,
            tc.tile_pool(name="attsb", bufs=1) as att_pool,
            tc.tile_pool(name="psum", bufs=6, space="PSUM") as psum_pool,
        ):
            w16_sb = const_pool.tile([128, NC16, HID], fp16)
            w8_sb = const_pool.tile([128, 2 * N8, HID], fp8)
            vb_sb = const_pool.tile([128, HID], fp16)
            att_sb = att_pool.tile([128, B_LOC * N_ST], f32)

            # small/const tensors on the ACT hwdge queue so they don't
            # serialize behind the enc stream on SP
            nc.scalar.dma_start(w16_sb[:], w16[:])
            nc.scalar.dma_start(w8_sb[:], w8[:])
            nc.scalar.dma_start(vb_sb[:], vb[:])

            cbb_sbs = [None] * B_LOC

            def load_cbb(b):
                t = cbb_pool.tile([128, HID], f32, tag="cbb", name=f"cbb_{b}")
                nc.scalar.dma_start(t[:], cbb[b])
                cbb_sbs[b] = t

            load_cbb(0)

            # warmup tanh for the ACT LUT-table dependency
            warm = const_pool.tile([128, 1], f32)
            nc.scalar.activation(
                warm[:], vb_sb[:, 0:1], mybir.ActivationFunctionType.Tanh
            )

            enc16_sbs = [None] * B_LOC
            enc8_sbs = [None] * B_LOC

            def alloc_row(b):
                enc16_sbs[b] = e16_pool.tile(
                    [128, NC16, SRC_LEN], fp16, tag="e16", name=f"e16_{b}"
                )
                enc8_sbs[b] = e8_pool.tile(
                    [128, 2 * N8, SRC_LEN], fp8, tag="e8", name=f"e8_{b}"
                )

            def load_row(b):
                nc.sync.dma_start(enc16_sbs[b][:], enc16[b])
                nc.sync.dma_start(enc8_sbs[b][:], enc8[b])

            # row 0 in s-slabs so the PE can start after the first slab
            alloc_row(0)
            for sg in range(N_SG):
                nc.sync.dma_start(
                    enc16_sbs[0][:, :, sg * SG:(sg + 1) * SG],
                    enc16[0, :, :, sg * SG:(sg + 1) * SG],
                )
                nc.sync.dma_start(
                    enc8_sbs[0][:, :, sg * SG:(sg + 1) * SG],
                    enc8[0, :, :, sg * SG:(sg + 1) * SG],
                )
            # row 1 goes straight after on the other buffer
            alloc_row(1)
            load_row(1)
            load_cbb(1)

            for b in range(B_LOC):
                for st in range(N_ST):
                    # prefetch row b+2 into the buffer row b just freed
                    if b + 2 < B_LOC and st == 2:
                        alloc_row(b + 2)
                        load_row(b + 2)
                        load_cbb(b + 2)
                    sl = slice(st * 128, (st + 1) * 128)
                    ps = [
                        psum_pool.tile(
                            [128, HG], f32, tag="ps", name=f"ps_{b}_{st}_{g}"
                        )
                        for g in range(N_HG)
                    ]

                    # split LdWeights/Matmult so the PE loads the next
                    # stationary while the current moving phase streams
                    def mm(psum, lhsT, rhs, start, stop, perf_mode=None):
                        nc.tensor.ldweights(lhsT, perf_mode=perf_mode)
                        inst = nc.tensor.matmul(
                            psum, lhsT=lhsT, rhs=rhs,
                            start=start, stop=stop, perf_mode=perf_mode,
                        )
                        inst.ins.ldweights = False

                    for hg in range(N_HG):
                        hsl = slice(hg * HG, (hg + 1) * HG)
                        for c in range(NC16):
                            mm(
                                ps[hg][:],
                                enc16_sbs[b][:, c, sl],
                                w16_sb[:, c, hsl],
                                start=(c == 0),
                                stop=False,
                            )
                        for j in range(N8):
                            mm(
                                ps[hg][:],
                                enc8_sbs[b][:, 2 * j:2 * j + 2, sl],
                                w8_sb[:, 2 * j:2 * j + 2, hsl],
                                start=False,
                                stop=(j == N8 - 1),
                                perf_mode=mybir.MatmulPerfMode.DoubleRow,
                            )
                    tanhE = te_pool.tile(
                        [128, HID], fp16, tag="te", name=f"te_{b}_{st}"
                    )
                    for hg in range(N_HG):
                        hsl = slice(hg * HG, (hg + 1) * HG)
                        pre = pre_pool.tile(
                            [128, HG], f32, tag="pre", name=f"pre_{b}_{st}_{hg}"
                        )
                        nc.vector.tensor_add(
                            out=pre[:], in0=ps[hg][:], in1=cbb_sbs[b][:, hsl]
                        )
                        nc.scalar.activation(
                            tanhE[:, hsl], pre[:],
                            mybir.ActivationFunctionType.Tanh,
                            scale=1.0 / WS,
                        )
                    tt = tt_pool.tile([128, HID], fp16, tag="tt", name=f"tt_{b}_{st}")
                    nc.vector.tensor_mul(out=tt[:], in0=tanhE[:], in1=vb_sb[:])
                    nc.vector.tensor_reduce(
                        att_sb[:, b * N_ST + st:b * N_ST + st + 1],
                        tt[:],
                        mybir.AxisListType.X,
                        mybir.AluOpType.add,
                    )
                nc.scalar.dma_start(att[b], att_sb[:, b * N_ST:(b + 1) * N_ST])
    nc.compile()
    return nc


def _get_nc():
    if "nc" not in _NC_CACHE:
        _NC_CACHE["nc"] = _build()
    return _NC_CACHE["nc"]


def kernel(hidden, encoder_outputs, attn_w, attn_b, v_w, _trace=False):
    hidden = np.asarray(hidden, dtype=np.float32)
    encoder_outputs = np.asarray(encoder_outputs, dtype=np.float32)
    attn_w = np.asarray(attn_w, dtype=np.float32)
    attn_b = np.asarray(attn_b, dtype=np.float32)
    v_w = np.asarray(v_w, dtype=np.float32)

    c_b = (hidden @ attn_w[:, :HID].T + attn_b) * WS   # [B, H] fp32, pre-scaled
    w_e = attn_w[:, HID:]                              # [H, E]
    # [E, H] -> [chunk, e, h] -> partition-major [e, chunk, h]
    w_t = (w_e.T * WS).reshape(N_EC, 128, HID)
    w16_dev = np.ascontiguousarray(
        w_t[:NC16].transpose(1, 0, 2)
    ).astype(np.float16)
    w8_dev = np.ascontiguousarray(
        w_t[NC16:].transpose(1, 0, 2)
    ).astype(ml_dtypes.float8_e4m3)
    vb_dev = np.ascontiguousarray(
        np.broadcast_to(v_w[None, :], (128, HID))
    ).astype(np.float16)

    nc = _get_nc()
    in_maps = []
    for core in range(N_CORES):
        b0 = core * B_LOC
        # enc[:, b, :] is [S, E]; make [e, chunk, s] lines per batch row
        e16_rows = np.empty((B_LOC, 128, NC16, SRC_LEN), dtype=np.float16)
        e8_rows = np.empty((B_LOC, 128, 2 * N8, SRC_LEN), dtype=ml_dtypes.float8_e4m3)
        for b in range(B_LOC):
            ect = encoder_outputs[:, b0 + b, :].T.reshape(N_EC, 128, SRC_LEN)
            e16_rows[b] = ect[:NC16].transpose(1, 0, 2)
            e8_rows[b] = ect[NC16:].transpose(1, 0, 2)
        cbb_dev = np.ascontiguousarray(
            np.broadcast_to(c_b[b0:b0 + B_LOC, None, :], (B_LOC, 128, HID))
        ).astype(np.float32)
        in_maps.append(
            {
                "enc16": e16_rows,
                "enc8": e8_rows,
                "w16": w16_dev,
                "w8": w8_dev,
                "cbb": cbb_dev,
                "vb": vb_dev,
            }
        )

    res = run_bass_kernel_spmd(
        nc, in_maps, core_ids=list(range(N_CORES)), trace=_trace
    )
    if _trace:
        _NC_CACHE["last_result"] = res

    att = np.concatenate(
        [
            res.results[c]["att"].transpose(0, 2, 1).reshape(B_LOC, SRC_LEN)
            for c in range(N_CORES)
        ],
        axis=0,
    )  # [B, S] logits

    m = att.max(axis=1, keepdims=True)
    e = np.exp(att - m)
    out = e / e.sum(axis=1, keepdims=True)
    return out.astype(np.float32)


# revision 6
# speedup vs baseline: 1.4261x; 1.1792x over previous
"""Trainium2 Bass kernel for nn_Attention (Bahdanau-style additive attention).

Reference computation:
    enc = encoder_outputs.transpose(1, 0, 2)            # [B, S, 2H]
    e_proj = enc @ w_e.T                                # [B, S, H]
    energy = tanh(h_proj[:, None, :] + e_proj + b)      # [B, S, H]
    att = energy @ v_w                                  # [B, S]
    out = softmax(att, axis=1)

Sharding: data-parallel over batch, 4 batch rows per core on 8 cores.

Per-core pipeline, mixed-precision contraction over E = 2048:
  - the first NC16*128 contraction rows run as fp16 matmuls (full
    precision), the remaining N8 pairs of 128-row chunks run as fp8
    (e4m3) matmuls in DoubleRow perf mode at 2x PE throughput
  - enc is pre-transposed and quantized on the host into partition-major
    [e, chunk, s] lines so every DMA is a plain (non-transpose) load
  - w_e and c_b are pre-scaled by WS=64 so the fp8 weights stay in the
    e4m3 normal range; the tanh activation applies scale=1/WS to undo it
  - epilogue per 128-position s-tile on the otherwise idle engines:
    DVE adds the host-precomputed broadcast bias c_b*WS, ACT applies
    tanh(x/WS), DVE multiplies by v_w and reduces over h into the logit
  - enc streams on the SP DMA queue (row 0 in four 512-position slabs
    so the PE starts early; rows 1-3 as single whole-row loads
    prefetched during the previous row), small tensors on the ACT queue
h_proj ([32,1024] @ [1024,1024]) and the final softmax over [32, 2048]
are tiny and run on the host in fp32.
"""

import sys

try:
    import concourse.bass as bass  # noqa: F401
except ImportError:
    sys.path.insert(0, "/opt/trn_rl_repo")

import numpy as np
import ml_dtypes

import concourse.bacc as bacc
import concourse.mybir as mybir
import concourse.tile as tile
from concourse.bass_utils import run_bass_kernel_spmd

HID = 1024
BATCH = 32
SRC_LEN = 2048

N_CORES = 8
B_LOC = BATCH // N_CORES      # 4
E = 2 * HID                   # 2048
N_EC = E // 128               # 16 e-chunks of 128
N8 = 5                        # fp8 DoubleRow chunk-pairs (2*N8 chunks)
NC16 = N_EC - 2 * N8          # fp16 chunks
N_ST = SRC_LEN // 128         # 16 s-tiles per batch row
HG = 512                      # h per psum bank
N_HG = HID // HG              # 2 h-groups
WS = 64.0                     # weight/bias pre-scale (fp8 range)
SG = 512                      # row-0 DMA slab width in s
N_SG = SRC_LEN // SG          # 4

f32 = mybir.dt.float32
fp16 = mybir.dt.float16
fp8 = mybir.dt.float8e4

_NC_CACHE = {}


def _build():
    nc = bacc.Bacc(
        "TRN2", target_bir_lowering=False, debug=False, num_devices=N_CORES
    )
    enc16 = nc.declare_dram_parameter(
        "enc16", [B_LOC, 128, NC16, SRC_LEN], fp16, isOutput=False
    )
    enc8 = nc.declare_dram_parameter(
        "enc8", [B_LOC, 128, 2 * N8, SRC_LEN], fp8, isOutput=False
    )
    w16 = nc.declare_dram_parameter("w16", [128, NC16, HID], fp16, isOutput=False)
    w8 = nc.declare_dram_parameter("w8", [128, 2 * N8, HID], fp8, isOutput=False)
    cbb = nc.declare_dram_parameter("cbb", [B_LOC, 128, HID], f32, isOutput=False)
    vb = nc.declare_dram_parameter("vb", [128, HID], fp16, isOutput=False)
    # [b, p, st]: logit(b, st*128 + p)
    att = nc.declare_dram_parameter("att", [B_LOC, 128, N_ST], f32, isOutput=True)

    with tile.TileContext(nc) as tc:
        with (
            tc.tile_pool(name="const", bufs=1) as const_pool,
            tc.tile_pool(name="e16p", bufs=2) as e16_pool,
            tc.tile_pool(name="e8p", bufs=2) as e8_pool,
            tc.tile_pool(name="cbbp", bufs=2) as cbb_pool,
            tc.tile_pool(name="prep", bufs=3) as pre_pool,
            tc.tile_pool(name="tep", bufs=3) as te_pool,
            tc.tile_pool(name="ttp", bufs=2) as tt_pool,
            tc.tile_pool(name="attsb", bufs=1) as att_pool,
            tc.tile_pool(name="psum", bufs=6, space="PSUM") as psum_pool,
        ):
            w16_sb = const_pool.tile([128, NC16, HID], fp16)
            w8_sb = const_pool.tile([128, 2 * N8, HID], fp8)
            vb_sb = const_pool.tile([128, HID], fp16)
            att_sb = att_pool.tile([128, B_LOC * N_ST], f32)

            # small/const tensors on the ACT hwdge queue so they don't
            # serialize behind the enc stream on SP; h-group 0 halves
            # first so the first matmul group gates on less data
            nc.scalar.dma_start(w16_sb[:, :, 0:HG], w16[:, :, 0:HG])
            nc.scalar.dma_start(w8_sb[:, :, 0:HG], w8[:, :, 0:HG])
            nc.scalar.dma_start(w16_sb[:, :, HG:HID], w16[:, :, HG:HID])
            nc.scalar.dma_start(w8_sb[:, :, HG:HID], w8[:, :, HG:HID])
            nc.scalar.dma_start(vb_sb[:], vb[:])

            cbb_sbs = [None] * B_LOC

            def load_cbb(b):
                t = cbb_pool.tile([128, HID], f32, tag="cbb", name=f"cbb_{b}")
                nc.scalar.dma_start(t[:], cbb[b])
                cbb_sbs[b] = t

            load_cbb(0)

            # warmup tanh for the ACT LUT-table dependency
            warm = const_pool.tile([128, 1], f32)
            nc.scalar.activation(
                warm[:], vb_sb[:, 0:1], mybir.ActivationFunctionType.Tanh
            )

            enc16_sbs = [None] * B_LOC
            enc8_sbs = [None] * B_LOC

            def alloc_row(b):
                enc16_sbs[b] = e16_pool.tile(
                    [128, NC16, SRC_LEN], fp16, tag="e16", name=f"e16_{b}"
                )
                enc8_sbs[b] = e8_pool.tile(
                    [128, 2 * N8, SRC_LEN], fp8, tag="e8", name=f"e8_{b}"
                )

            def load_row(b):
                nc.sync.dma_start(enc16_sbs[b][:], enc16[b])
                nc.sync.dma_start(enc8_sbs[b][:], enc8[b])

            # row 0 in graduated s-slabs so the PE can start early
            alloc_row(0)
            s0 = 0
            for sw in (256, 256, 512, 1024):
                nc.sync.dma_start(
                    enc16_sbs[0][:, :, s0:s0 + sw],
                    enc16[0, :, :, s0:s0 + sw],
                )
                nc.sync.dma_start(
                    enc8_sbs[0][:, :, s0:s0 + sw],
                    enc8[0, :, :, s0:s0 + sw],
                )
                s0 += sw

            for b in range(B_LOC):
                for st in range(N_ST):
                    # prefetch the next row once this row is under way so
                    # its DMA doesn't compete with the startup-critical loads
                    if b + 1 < B_LOC and st == 2:
                        alloc_row(b + 1)
                        load_row(b + 1)
                        load_cbb(b + 1)
                    sl = slice(st * 128, (st + 1) * 128)
                    ps = [
                        psum_pool.tile(
                            [128, HG], f32, tag="ps", name=f"ps_{b}_{st}_{g}"
                        )
                        for g in range(N_HG)
                    ]

                    # split LdWeights/Matmult so the PE loads the next
                    # stationary while the current moving phase streams
                    def mm(psum, lhsT, rhs, start, stop, perf_mode=None):
                        nc.tensor.ldweights(lhsT, perf_mode=perf_mode)
                        inst = nc.tensor.matmul(
                            psum, lhsT=lhsT, rhs=rhs,
                            start=start, stop=stop, perf_mode=perf_mode,
                        )
                        inst.ins.ldweights = False

                    for hg in range(N_HG):
                        hsl = slice(hg * HG, (hg + 1) * HG)
                        for c in range(NC16):
                            mm(
                                ps[hg][:],
                                enc16_sbs[b][:, c, sl],
                                w16_sb[:, c, hsl],
                                start=(c == 0),
                                stop=False,
                            )
                        for j in range(N8):
                            mm(
                                ps[hg][:],
                                enc8_sbs[b][:, 2 * j:2 * j + 2, sl],
                                w8_sb[:, 2 * j:2 * j + 2, hsl],
                                start=False,
                                stop=(j == N8 - 1),
                                perf_mode=mybir.MatmulPerfMode.DoubleRow,
                            )
                    tanhE = te_pool.tile(
                        [128, HID], fp16, tag="te", name=f"te_{b}_{st}"
                    )
                    for hg in range(N_HG):
                        hsl = slice(hg * HG, (hg + 1) * HG)
                        pre = pre_pool.tile(
                            [128, HG], f32, tag="pre", name=f"pre_{b}_{st}_{hg}"
                        )
                        nc.vector.tensor_add(
                            out=pre[:], in0=ps[hg][:], in1=cbb_sbs[b][:, hsl]
                        )
                        nc.scalar.activation(
                            tanhE[:, hsl], pre[:],
                            mybir.ActivationFunctionType.Tanh,
                            scale=1.0 / WS,
                        )
                    tt = tt_pool.tile([128, HID], fp16, tag="tt", name=f"tt_{b}_{st}")
                    nc.vector.tensor_mul(out=tt[:], in0=tanhE[:], in1=vb_sb[:])
                    nc.vector.tensor_reduce(
                        att_sb[:, b * N_ST + st:b * N_ST + st + 1],
                        tt[:],
                        mybir.AxisListType.X,
                        mybir.AluOpType.add,
                    )
                nc.scalar.dma_start(att[b], att_sb[:, b * N_ST:(b + 1) * N_ST])
    nc.compile()
    return nc


def _get_nc():
    if "nc" not in _NC_CACHE:
        _NC_CACHE["nc"] = _build()
    return _NC_CACHE["nc"]


def kernel(hidden, encoder_outputs, attn_w, attn_b, v_w, _trace=False):
    hidden = np.asarray(hidden, dtype=np.float32)
    encoder_outputs = np.asarray(encoder_outputs, dtype=np.float32)
    attn_w = np.asarray(attn_w, dtype=np.float32)
    attn_b = np.asarray(attn_b, dtype=np.float32)
    v_w = np.asarray(v_w, dtype=np.float32)

    c_b = (hidden @ attn_w[:, :HID].T + attn_b) * WS   # [B, H] fp32, pre-scaled
    w_e = attn_w[:, HID:]                              # [H, E]
    # [E, H] -> [chunk, e, h] -> partition-major [e, chunk, h]
    w_t = (w_e.T * WS).reshape(N_EC, 128, HID)
    w16_dev = np.ascontiguousarray(
        w_t[:NC16].transpose(1, 0, 2)
    ).astype(np.float16)
    w8_dev = np.ascontiguousarray(
        w_t[NC16:].transpose(1, 0, 2)
    ).astype(ml_dtypes.float8_e4m3)
    vb_dev = np.ascontiguousarray(
        np.broadcast_to(v_w[None, :], (128, HID))
    ).astype(np.float16)

    nc = _get_nc()
    in_maps = []
    for core in range(N_CORES):
        b0 = core * B_LOC
        # enc[:, b, :] is [S, E]; make [e, chunk, s] lines per batch row
        e16_rows = np.empty((B_LOC, 128, NC16, SRC_LEN), dtype=np.float16)
        e8_rows = np.empty((B_LOC, 128, 2 * N8, SRC_LEN), dtype=ml_dtypes.float8_e4m3)
        for b in range(B_LOC):
            ect = encoder_outputs[:, b0 + b, :].T.reshape(N_EC, 128, SRC_LEN)
            e16_rows[b] = ect[:NC16].transpose(1, 0, 2)
            e8_rows[b] = ect[NC16:].transpose(1, 0, 2)
        cbb_dev = np.ascontiguousarray(
            np.broadcast_to(c_b[b0:b0 + B_LOC, None, :], (B_LOC, 128, HID))
        ).astype(np.float32)
        in_maps.append(
            {
                "enc16": e16_rows,
                "enc8": e8_rows,
                "w16": w16_dev,
                "w8": w8_dev,
                "cbb": cbb_dev,
                "vb": vb_dev,
            }
        )

    res = run_bass_kernel_spmd(
        nc, in_maps, core_ids=list(range(N_CORES)), trace=_trace
    )
    if _trace:
        _NC_CACHE["last_result"] = res

    att = np.concatenate(
        [
            res.results[c]["att"].transpose(0, 2, 1).reshape(B_LOC, SRC_LEN)
            for c in range(N_CORES)
        ],
        axis=0,
    )  # [B, S] logits

    m = att.max(axis=1, keepdims=True)
    e = np.exp(att - m)
    out = e / e.sum(axis=1, keepdims=True)
    return out.astype(np.float32)


# revision 8
# speedup vs baseline: 1.4276x; 1.0010x over previous
"""Trainium2 Bass kernel for nn_Attention (Bahdanau-style additive attention).

Reference computation:
    enc = encoder_outputs.transpose(1, 0, 2)            # [B, S, 2H]
    e_proj = enc @ w_e.T                                # [B, S, H]
    energy = tanh(h_proj[:, None, :] + e_proj + b)      # [B, S, H]
    att = energy @ v_w                                  # [B, S]
    out = softmax(att, axis=1)

Sharding: data-parallel over batch, 4 batch rows per core on 8 cores.

Per-core pipeline, mixed-precision contraction over E = 2048:
  - the first NC16*128 contraction rows run as fp16 matmuls (full
    precision), the remaining N8 pairs of 128-row chunks run as fp8
    (e4m3) matmuls in DoubleRow perf mode at 2x PE throughput
  - enc is pre-transposed and quantized on the host into partition-major
    [e, chunk, s] lines so every DMA is a plain (non-transpose) load
  - w_e and c_b are pre-scaled by WS=64 so the fp8 weights stay in the
    e4m3 normal range; the tanh activation applies scale=1/WS to undo it
  - epilogue per 128-position s-tile on the otherwise idle engines:
    DVE adds the host-precomputed broadcast bias c_b*WS, ACT applies
    tanh(x/WS), DVE multiplies by v_w and reduces over h into the logit
  - enc streams on the SP DMA queue (row 0 in four 512-position slabs
    so the PE starts early; rows 1-3 as single whole-row loads
    prefetched during the previous row), small tensors on the ACT queue
h_proj ([32,1024] @ [1024,1024]) and the final softmax over [32, 2048]
are tiny and run on the host in fp32.
"""

import sys

try:
    import concourse.bass as bass  # noqa: F401
except ImportError:
    sys.path.insert(0, "/opt/trn_rl_repo")

import numpy as np
import ml_dtypes

import concourse.bacc as bacc
import concourse.mybir as mybir
import concourse.tile as tile
from concourse.bass_utils import run_bass_kernel_spmd

HID = 1024
BATCH = 32
SRC_LEN = 2048

N_CORES = 8
B_LOC = BATCH // N_CORES      # 4
E = 2 * HID                   # 2048
N_EC = E // 128               # 16 e-chunks of 128
N8 = 5                        # fp8 DoubleRow chunk-pairs (2*N8 chunks)
NC16 = N_EC - 2 * N8          # fp16 chunks
N_ST = SRC_LEN // 128         # 16 s-tiles per batch row
HG = 512                      # h per psum bank
N_HG = HID // HG              # 2 h-groups
WS = 64.0                     # weight/bias pre-scale (fp8 range)
SG = 512                      # row-0 DMA slab width in s
N_SG = SRC_LEN // SG          # 4

f32 = mybir.dt.float32
fp16 = mybir.dt.float16
fp8 = mybir.dt.float8e4

_NC_CACHE = {}


def _build():
    nc = bacc.Bacc(
        "TRN2", target_bir_lowering=False, debug=False, num_devices=N_CORES
    )
    enc16 = nc.declare_dram_parameter(
        "enc16", [B_LOC, 128, NC16, SRC_LEN], fp16, isOutput=False
    )
    enc8 = nc.declare_dram_parameter(
        "enc8", [B_LOC, 128, 2 * N8, SRC_LEN], fp8, isOutput=False
    )
    w16 = nc.declare_dram_parameter("w16", [128, NC16, HID], fp16, isOutput=False)
    w8 = nc.declare_dram_parameter("w8", [128, 2 * N8, HID], fp8, isOutput=False)
    cbb = nc.declare_dram_parameter("cbb", [B_LOC, 128, HID], f32, isOutput=False)
    vb = nc.declare_dram_parameter("vb", [128, HID], fp16, isOutput=False)
    # [b, p, st]: logit(b, st*128 + p)
    att = nc.declare_dram_parameter("att", [B_LOC, 128, N_ST], f32, isOutput=True)

    with tile.TileContext(nc) as tc:
        with (
            tc.tile_pool(name="const", bufs=1) as const_pool,
            tc.tile_pool(name="e16p", bufs=2) as e16_pool,
            tc.tile_pool(name="e8p", bufs=2) as e8_pool,
            tc.tile_pool(name="cbbp", bufs=2) as cbb_pool,
            tc.tile_pool(name="prep", bufs=3) as pre_pool,
            tc.tile_pool(name="tep", bufs=3) as te_pool,
            tc.tile_pool(name="ttp", bufs=2) as tt_pool,
            tc.tile_pool(name="attsb", bufs=1) as att_pool,
            tc.tile_pool(name="psum", bufs=6, space="PSUM") as psum_pool,
        ):
            w16_sb = const_pool.tile([128, NC16, HID], fp16)
            w8_sb = const_pool.tile([128, 2 * N8, HID], fp8)
            vb_sb = const_pool.tile([128, HID], fp16)
            att_sb = att_pool.tile([128, B_LOC * N_ST], f32)

            # small/const tensors on the ACT hwdge queue so they don't
            # serialize behind the enc stream on SP; staged in matmul
            # consumption order (h-group 0 chunks first) so the first
            # group's instructions gate on as little data as possible
            for hg in range(N_HG):
                hsl = slice(hg * HG, (hg + 1) * HG)
                for c in range(NC16):
                    nc.scalar.dma_start(w16_sb[:, c, hsl], w16[:, c, hsl])
                for j in range(N8):
                    jsl = slice(2 * j, 2 * j + 2)
                    nc.scalar.dma_start(w8_sb[:, jsl, hsl], w8[:, jsl, hsl])
            nc.scalar.dma_start(vb_sb[:], vb[:])

            cbb_sbs = [None] * B_LOC

            def load_cbb(b):
                t = cbb_pool.tile([128, HID], f32, tag="cbb", name=f"cbb_{b}")
                nc.scalar.dma_start(t[:], cbb[b])
                cbb_sbs[b] = t

            load_cbb(0)

            # warmup tanh for the ACT LUT-table dependency
            warm = const_pool.tile([128, 1], f32)
            nc.scalar.activation(
                warm[:], vb_sb[:, 0:1], mybir.ActivationFunctionType.Tanh
            )

            enc16_sbs = [None] * B_LOC
            enc8_sbs = [None] * B_LOC

            def alloc_row(b):
                enc16_sbs[b] = e16_pool.tile(
                    [128, NC16, SRC_LEN], fp16, tag="e16", name=f"e16_{b}"
                )
                enc8_sbs[b] = e8_pool.tile(
                    [128, 2 * N8, SRC_LEN], fp8, tag="e8", name=f"e8_{b}"
                )

            def load_row(b):
                nc.sync.dma_start(enc16_sbs[b][:], enc16[b])
                nc.sync.dma_start(enc8_sbs[b][:], enc8[b])

            # row 0 in graduated s-slabs so the PE can start early
            alloc_row(0)
            s0 = 0
            for sw in (256, 256, 512, 1024):
                nc.sync.dma_start(
                    enc16_sbs[0][:, :, s0:s0 + sw],
                    enc16[0, :, :, s0:s0 + sw],
                )
                nc.sync.dma_start(
                    enc8_sbs[0][:, :, s0:s0 + sw],
                    enc8[0, :, :, s0:s0 + sw],
                )
                s0 += sw

            for b in range(B_LOC):
                for st in range(N_ST):
                    # prefetch the next row once this row is under way so
                    # its DMA doesn't compete with the startup-critical loads
                    if b + 1 < B_LOC and st == 2:
                        alloc_row(b + 1)
                        load_row(b + 1)
                        load_cbb(b + 1)
                    sl = slice(st * 128, (st + 1) * 128)
                    ps = [
                        psum_pool.tile(
                            [128, HG], f32, tag="ps", name=f"ps_{b}_{st}_{g}"
                        )
                        for g in range(N_HG)
                    ]

                    # split LdWeights/Matmult so the PE loads the next
                    # stationary while the current moving phase streams;
                    # one LdWeights (the enc chunk) feeds both h-groups
                    def mm_pair(lhsT, w_sb_c, start, stop, perf_mode=None):
                        nc.tensor.ldweights(lhsT, perf_mode=perf_mode)
                        for hg in range(N_HG):
                            inst = nc.tensor.matmul(
                                ps[hg][:],
                                lhsT=lhsT,
                                rhs=w_sb_c[:, hg * HG:(hg + 1) * HG]
                                if perf_mode is None
                                else w_sb_c[:, :, hg * HG:(hg + 1) * HG],
                                start=start, stop=stop, perf_mode=perf_mode,
                            )
                            inst.ins.ldweights = False

                    for c in range(NC16):
                        mm_pair(
                            enc16_sbs[b][:, c, sl],
                            w16_sb[:, c],
                            start=(c == 0),
                            stop=False,
                        )
                    for j in range(N8):
                        mm_pair(
                            enc8_sbs[b][:, 2 * j:2 * j + 2, sl],
                            w8_sb[:, 2 * j:2 * j + 2],
                            start=False,
                            stop=(j == N8 - 1),
                            perf_mode=mybir.MatmulPerfMode.DoubleRow,
                        )
                    tanhE = te_pool.tile(
                        [128, HID], fp16, tag="te", name=f"te_{b}_{st}"
                    )
                    for hg in range(N_HG):
                        hsl = slice(hg * HG, (hg + 1) * HG)
                        pre = pre_pool.tile(
                            [128, HG], f32, tag="pre", name=f"pre_{b}_{st}_{hg}"
                        )
                        nc.vector.tensor_add(
                            out=pre[:], in0=ps[hg][:], in1=cbb_sbs[b][:, hsl]
                        )
                        nc.scalar.activation(
                            tanhE[:, hsl], pre[:],
                            mybir.ActivationFunctionType.Tanh,
                            scale=1.0 / WS,
                        )
                    tt = tt_pool.tile([128, HID], fp16, tag="tt", name=f"tt_{b}_{st}")
                    nc.vector.tensor_mul(out=tt[:], in0=tanhE[:], in1=vb_sb[:])
                    nc.vector.tensor_reduce(
                        att_sb[:, b * N_ST + st:b * N_ST + st + 1],
                        tt[:],
                        mybir.AxisListType.X,
                        mybir.AluOpType.add,
                    )
                nc.scalar.dma_start(att[b], att_sb[:, b * N_ST:(b + 1) * N_ST])
    nc.compile()
    return nc


def _get_nc():
    if "nc" not in _NC_CACHE:
        _NC_CACHE["nc"] = _build()
    return _NC_CACHE["nc"]


def kernel(hidden, encoder_outputs, attn_w, attn_b, v_w, _trace=False):
    hidden = np.asarray(hidden, dtype=np.float32)
    encoder_outputs = np.asarray(encoder_outputs, dtype=np.float32)
    attn_w = np.asarray(attn_w, dtype=np.float32)
    attn_b = np.asarray(attn_b, dtype=np.float32)
    v_w = np.asarray(v_w, dtype=np.float32)

    c_b = (hidden @ attn_w[:, :HID].T + attn_b) * WS   # [B, H] fp32, pre-scaled
    w_e = attn_w[:, HID:]                              # [H, E]
    # [E, H] -> [chunk, e, h] -> partition-major [e, chunk, h]
    w_t = (w_e.T * WS).reshape(N_EC, 128, HID)
    w16_dev = np.ascontiguousarray(
        w_t[:NC16].transpose(1, 0, 2)
    ).astype(np.float16)
    w8_dev = np.ascontiguousarray(
        w_t[NC16:].transpose(1, 0, 2)
    ).astype(ml_dtypes.float8_e4m3)
    vb_dev = np.ascontiguousarray(
        np.broadcast_to(v_w[None, :], (128, HID))
    ).astype(np.float16)

    nc = _get_nc()
    in_maps = []
    for core in range(N_CORES):
        b0 = core * B_LOC
        # enc[:, b, :] is [S, E]; make [e, chunk, s] lines per batch row
        e16_rows = np.empty((B_LOC, 128, NC16, SRC_LEN), dtype=np.float16)
        e8_rows = np.empty((B_LOC, 128, 2 * N8, SRC_LEN), dtype=ml_dtypes.float8_e4m3)
        for b in range(B_LOC):
            ect = encoder_outputs[:, b0 + b, :].T.reshape(N_EC, 128, SRC_LEN)
            e16_rows[b] = ect[:NC16].transpose(1, 0, 2)
            e8_rows[b] = ect[NC16:].transpose(1, 0, 2)
        cbb_dev = np.ascontiguousarray(
            np.broadcast_to(c_b[b0:b0 + B_LOC, None, :], (B_LOC, 128, HID))
        ).astype(np.float32)
        in_maps.append(
            {
                "enc16": e16_rows,
                "enc8": e8_rows,
                "w16": w16_dev,
                "w8": w8_dev,
                "cbb": cbb_dev,
                "vb": vb_dev,
            }
        )

    res = run_bass_kernel_spmd(
        nc, in_maps, core_ids=list(range(N_CORES)), trace=_trace
    )
    if _trace:
        _NC_CACHE["last_result"] = res

    att = np.concatenate(
        [
            res.results[c]["att"].transpose(0, 2, 1).reshape(B_LOC, SRC_LEN)
            for c in range(N_CORES)
        ],
        axis=0,
    )  # [B, S] logits

    m = att.max(axis=1, keepdims=True)
    e = np.exp(att - m)
    out = e / e.sum(axis=1, keepdims=True)
    return out.astype(np.float32)


# revision 11
# speedup vs baseline: 1.5620x; 1.0942x over previous
"""Trainium2 Bass kernel for nn_Attention (Bahdanau-style additive attention).

Reference computation:
    enc = encoder_outputs.transpose(1, 0, 2)            # [B, S, 2H]
    e_proj = enc @ w_e.T                                # [B, S, H]
    energy = tanh(h_proj[:, None, :] + e_proj + b)      # [B, S, H]
    att = energy @ v_w                                  # [B, S]
    out = softmax(att, axis=1)

Sharding: data-parallel over batch, 4 batch rows per core on 8 cores.

Per-core pipeline, |v|-stratified mixed precision: the logit error from
quantizing the e_proj GEMM is sum_h v_h * tanh' * dx_h, so the h
columns are permuted by descending |v_h| (host side) and the HOT
highest-|v| columns are computed in fp16 while the remaining COLD
columns run entirely in fp8 (e4m3) DoubleRow matmuls at 2x PE
throughput.  This buys the same accuracy as a chunk-wise fp16/fp8
split at ~12% less PE time.
  - enc is pre-transposed and quantized on the host into partition-major
    [e, chunk, s] lines (both dtypes cover all 16 contraction chunks),
    streamed per half batch row; all DMA is plain loads, no transpose
  - w_e and c_b are pre-scaled by WS=64 so the fp8 weights stay in the
    e4m3 normal range; the tanh activation applies scale=1/WS to undo it
  - LdWeights is split from Matmult so the next stationary (an enc
    chunk) loads while the current moving phase streams
  - epilogue per 128-position s-tile: DVE adds the broadcast bias
    c_b*WS per psum region, ACT applies tanh(x/WS), GPSIMD (otherwise
    idle) multiplies by v and reduces over h into the logit column
h_proj ([32,1024] @ [1024,1024]) and the final softmax over [32, 2048]
are tiny and run on the host in fp32.
"""

import sys

try:
    import concourse.bass as bass  # noqa: F401
except ImportError:
    sys.path.insert(0, "/opt/trn_rl_repo")

import numpy as np
import ml_dtypes

import concourse.bacc as bacc
import concourse.mybir as mybir
import concourse.tile as tile
from concourse.bass_utils import run_bass_kernel_spmd

HID = 1024
BATCH = 32
SRC_LEN = 2048

N_CORES = 8
B_LOC = BATCH // N_CORES      # 4
E = 2 * HID                   # 2048
N_EC = E // 128               # 16 e-chunks of 128
N_DR = N_EC // 2              # 8 fp8 DoubleRow chunk-pairs
HOT = 192                     # fp16 h-columns (highest |v|), permuted first
COLD = HID - HOT              # 832 fp8 h-columns
C0 = 512                      # cold psum region split: 512 + 320
C1 = COLD - C0                # 320
SH = SRC_LEN // 2             # 1024 s per half-row pipeline stage
N_STH = SH // 128             # 8 s-tiles per half
WS = 64.0                     # weight/bias pre-scale (fp8 range)

f32 = mybir.dt.float32
fp16 = mybir.dt.float16
fp8 = mybir.dt.float8e4

_NC_CACHE = {}


def _build():
    nc = bacc.Bacc(
        "TRN2", target_bir_lowering=False, debug=False, num_devices=N_CORES
    )
    enc16 = nc.declare_dram_parameter(
        "enc16", [B_LOC, 128, N_EC, SRC_LEN], fp16, isOutput=False
    )
    enc8 = nc.declare_dram_parameter(
        "enc8", [B_LOC, 128, N_EC, SRC_LEN], fp8, isOutput=False
    )
    w16 = nc.declare_dram_parameter("w16", [128, N_EC, HOT], fp16, isOutput=False)
    w8 = nc.declare_dram_parameter("w8", [128, N_EC, COLD], fp8, isOutput=False)
    cbb = nc.declare_dram_parameter("cbb", [B_LOC, 128, HID], f32, isOutput=False)
    vb = nc.declare_dram_parameter("vb", [128, HID], fp16, isOutput=False)
    # [b, p, st]: logit(b, st*128 + p) in permuted-h space (h only summed)
    att = nc.declare_dram_parameter(
        "att", [B_LOC, 128, SRC_LEN // 128], f32, isOutput=True
    )

    with tile.TileContext(nc) as tc:
        with (
            tc.tile_pool(name="const", bufs=1) as const_pool,
            tc.tile_pool(name="e16p", bufs=2) as e16_pool,
            tc.tile_pool(name="e8p", bufs=2) as e8_pool,
            tc.tile_pool(name="cbbp", bufs=2) as cbb_pool,
            tc.tile_pool(name="prep", bufs=4) as pre_pool,
            tc.tile_pool(name="tep", bufs=3) as te_pool,
            tc.tile_pool(name="ttp", bufs=2) as tt_pool,
            tc.tile_pool(name="attsb", bufs=1) as att_pool,
            tc.tile_pool(name="psum", bufs=2, space="PSUM") as psum_pool,
        ):
            w16_sb = const_pool.tile([128, N_EC, HOT], fp16)
            w8_sb = const_pool.tile([128, N_EC, COLD], fp8)
            vb_sb = const_pool.tile([128, HID], fp16)
            att_sb = att_pool.tile([128, BATCH // N_CORES * (SRC_LEN // 128)], f32)

            # consts on the ACT hwdge queue, staged in first-consumption
            # order (hot chunks first) so the first group gates minimally
            for c in range(N_EC):
                nc.scalar.dma_start(w16_sb[:, c], w16[:, c])
            for j in range(N_DR):
                jsl = slice(2 * j, 2 * j + 2)
                nc.scalar.dma_start(w8_sb[:, jsl], w8[:, jsl])
            nc.scalar.dma_start(vb_sb[:], vb[:])

            cbb_sbs = [None] * B_LOC

            def load_cbb(b):
                t = cbb_pool.tile([128, HID], f32, tag="cbb", name=f"cbb_{b}")
                nc.scalar.dma_start(t[:], cbb[b])
                cbb_sbs[b] = t

            load_cbb(0)

            # warmup tanh for the ACT LUT-table dependency
            warm = const_pool.tile([128, 1], f32)
            nc.scalar.activation(
                warm[:], vb_sb[:, 0:1], mybir.ActivationFunctionType.Tanh
            )

            halves = [(b, h) for b in range(B_LOC) for h in range(2)]
            e16_sbs = {}
            e8_sbs = {}

            def alloc_half(i):
                e16_sbs[i] = e16_pool.tile(
                    [128, N_EC, SH], fp16, tag="e16", name=f"e16_{i}"
                )
                e8_sbs[i] = e8_pool.tile(
                    [128, N_EC, SH], fp8, tag="e8", name=f"e8_{i}"
                )

            def load_half(i):
                b, h = halves[i]
                ssl = slice(h * SH, (h + 1) * SH)
                nc.sync.dma_start(e16_sbs[i][:], enc16[b, :, :, ssl])
                nc.sync.dma_start(e8_sbs[i][:], enc8[b, :, :, ssl])

            # first half in graduated s-slabs so the PE starts early
            alloc_half(0)
            s0 = 0
            for sw in (256, 256, 512):
                nc.sync.dma_start(
                    e16_sbs[0][:, :, s0:s0 + sw], enc16[0, :, :, s0:s0 + sw]
                )
                nc.sync.dma_start(
                    e8_sbs[0][:, :, s0:s0 + sw], enc8[0, :, :, s0:s0 + sw]
                )
                s0 += sw

            for i, (b, h) in enumerate(halves):
                for st in range(N_STH):
                    if i + 1 < len(halves) and st == 1:
                        alloc_half(i + 1)
                        load_half(i + 1)
                        if h == 1:
                            load_cbb(b + 1)
                    sl = slice(st * 128, (st + 1) * 128)
                    ps_h = psum_pool.tile(
                        [128, HOT], f32, tag="psh", name=f"psh_{i}_{st}"
                    )
                    ps_c0 = psum_pool.tile(
                        [128, C0], f32, tag="psc0", name=f"psc0_{i}_{st}"
                    )
                    ps_c1 = psum_pool.tile(
                        [128, C1], f32, tag="psc1", name=f"psc1_{i}_{st}"
                    )

                    # split LdWeights/Matmult: the next stationary (enc
                    # chunk) loads while the current moving phase streams
                    def mm(psum, rhs, start, stop, perf_mode=None):
                        inst = nc.tensor.matmul(
                            psum, lhsT=lhs, rhs=rhs,
                            start=start, stop=stop, perf_mode=perf_mode,
                        )
                        inst.ins.ldweights = False

                    for c in range(N_EC):
                        lhs = e16_sbs[i][:, c, sl]
                        nc.tensor.ldweights(lhs)
                        mm(ps_h[:], w16_sb[:, c], start=(c == 0),
                           stop=(c == N_EC - 1))
                    for j in range(N_DR):
                        lhs = e8_sbs[i][:, 2 * j:2 * j + 2, sl]
                        nc.tensor.ldweights(
                            lhs, perf_mode=mybir.MatmulPerfMode.DoubleRow
                        )
                        mm(ps_c0[:], w8_sb[:, 2 * j:2 * j + 2, 0:C0],
                           start=(j == 0), stop=(j == N_DR - 1),
                           perf_mode=mybir.MatmulPerfMode.DoubleRow)
                        mm(ps_c1[:], w8_sb[:, 2 * j:2 * j + 2, C0:COLD],
                           start=(j == 0), stop=(j == N_DR - 1),
                           perf_mode=mybir.MatmulPerfMode.DoubleRow)

                    tanhE = te_pool.tile(
                        [128, HID], fp16, tag="te", name=f"te_{i}_{st}"
                    )
                    for ps, lo, hi in (
                        (ps_h, 0, HOT),
                        (ps_c0, HOT, HOT + C0),
                        (ps_c1, HOT + C0, HID),
                    ):
                        pre = pre_pool.tile(
                            [128, hi - lo], f32, tag="pre",
                            name=f"pre_{i}_{st}_{lo}",
                        )
                        nc.vector.tensor_add(
                            out=pre[:], in0=ps[:], in1=cbb_sbs[b][:, lo:hi]
                        )
                        nc.scalar.activation(
                            tanhE[:, lo:hi], pre[:],
                            mybir.ActivationFunctionType.Tanh,
                            scale=1.0 / WS,
                        )
                    tt = tt_pool.tile(
                        [128, HID], fp16, tag="tt", name=f"tt_{i}_{st}"
                    )
                    nc.gpsimd.tensor_mul(out=tt[:], in0=tanhE[:], in1=vb_sb[:])
                    k = b * (SRC_LEN // 128) + h * N_STH + st
                    nc.vector.tensor_reduce(
                        att_sb[:, k:k + 1],
                        tt[:],
                        mybir.AxisListType.X,
                        mybir.AluOpType.add,
                    )
                if h == 1:
                    nst = SRC_LEN // 128
                    nc.scalar.dma_start(
                        att[b], att_sb[:, b * nst:(b + 1) * nst]
                    )
    nc.compile()
    return nc


def _get_nc():
    if "nc" not in _NC_CACHE:
        _NC_CACHE["nc"] = _build()
    return _NC_CACHE["nc"]


def kernel(hidden, encoder_outputs, attn_w, attn_b, v_w, _trace=False):
    hidden = np.asarray(hidden, dtype=np.float32)
    encoder_outputs = np.asarray(encoder_outputs, dtype=np.float32)
    attn_w = np.asarray(attn_w, dtype=np.float32)
    attn_b = np.asarray(attn_b, dtype=np.float32)
    v_w = np.asarray(v_w, dtype=np.float32)

    perm = np.argsort(-np.abs(v_w))                    # hot |v| first
    c_b = ((hidden @ attn_w[:, :HID].T + attn_b)[:, perm]) * WS
    w_e = attn_w[:, HID:][perm]                        # [H, E] permuted rows
    # [E, H] -> [chunk, e, h] -> partition-major [e, chunk, h]
    w_t = np.ascontiguousarray(
        (w_e.T * WS).reshape(N_EC, 128, HID).transpose(1, 0, 2)
    )
    w16_dev = w_t[:, :, :HOT].astype(np.float16)
    w8_dev = np.ascontiguousarray(w_t[:, :, HOT:]).astype(ml_dtypes.float8_e4m3)
    vb_dev = np.ascontiguousarray(
        np.broadcast_to(v_w[perm][None, :], (128, HID))
    ).astype(np.float16)

    nc = _get_nc()
    in_maps = []
    for core in range(N_CORES):
        b0 = core * B_LOC
        e16_rows = np.empty((B_LOC, 128, N_EC, SRC_LEN), dtype=np.float16)
        e8_rows = np.empty((B_LOC, 128, N_EC, SRC_LEN), dtype=ml_dtypes.float8_e4m3)
        for b in range(B_LOC):
            # enc[:, b, :] is [S, E]; make [e, chunk, s] lines
            ect = encoder_outputs[:, b0 + b, :].T.reshape(N_EC, 128, SRC_LEN)
            ect = ect.transpose(1, 0, 2)
            e16_rows[b] = ect
            e8_rows[b] = ect
        cbb_dev = np.ascontiguousarray(
            np.broadcast_to(c_b[b0:b0 + B_LOC, None, :], (B_LOC, 128, HID))
        ).astype(np.float32)
        in_maps.append(
            {
                "enc16": e16_rows,
                "enc8": e8_rows,
                "w16": w16_dev,
                "w8": w8_dev,
                "cbb": cbb_dev,
                "vb": vb_dev,
            }
        )

    res = run_bass_kernel_spmd(
        nc, in_maps, core_ids=list(range(N_CORES)), trace=_trace
    )
    if _trace:
        _NC_CACHE["last_result"] = res

    att = np.concatenate(
        [
            res.results[c]["att"].transpose(0, 2, 1).reshape(B_LOC, SRC_LEN)
            for c in range(N_CORES)
        ],
        axis=0,
    )  # [B, S] logits

    m = att.max(axis=1, keepdims=True)
    e = np.exp(att - m)
    out = e / e.sum(axis=1, keepdims=True)
    return out.astype(np.float32)
